# revision 25
# baseline (speedup 1.0000x reference)
"""Trainium2 Bass kernel for nn_EuclideanEmbedding (vq_codebook).

reference:
    distances = cdist(x, p)                      # (8192, 512)
    r1 = mean_j min_i distances[i, j]            # scalar
    r2 = mean_i min_j distances[i, j]            # scalar

Strategy (8 NeuronCores):
  - Shard x along batch: 1024 rows/core; replicate the (512, 64) codebook.
  - Per core, squared distances via float32r matmuls (1 row/cycle on the
    PE vs 4 for plain fp32), one per 128-row tile, K = 67:
      D2 = [x, 1, hi, lo] @ [-2 p, ||p||^2, 1, 1]^T
    where hi = bf16(||x||^2), lo = ||x||^2 - hi.  f32r rounds operands to
    ~11 mantissa bits; splitting the large ||x||^2 term into a bf16 part
    (exactly f32r-representable) plus a small residual keeps the D2
    error at the x.p noise floor instead of ||x||^2 * 2^-12.
  - Tiles are processed in pairs ([128, 2, 512] PSUM supertiles);
    ScalarE takes sqrt (PSUM->SBUF, one op per pair); DMA streams the
    distance shard out (contiguous 256KB blocks).
  - VectorE r2 partials: per-row min over the codebook on PSUM D2.
  - VectorE r1 partials: a SECOND set of matmuls emits the transposed
    layout D2T[j, batch] (the tensor engine has spare capacity), so the
    min-over-batch is also a free-dim reduce - no elementwise min chain,
    no on-chip transposes. Only a [128, 4] vector leaves each core.
  - Host combines the per-core partial minima (the cross-device min /
    mean all-reduce of the sharding hint) while unsharding.
"""

import numpy as np

import concourse.bacc as bacc
import concourse.tile as tile
from concourse import mybir
from concourse.bass_utils import run_bass_kernel_spmd

BATCH = 8192
NV = 512          # codebook vectors
D = 64            # latent dim
NCORES = 8
BPC = BATCH // NCORES   # 1024 batch rows per core
P = 128                 # partitions
NTILES = BPC // P       # 8 tiles of 128 batch rows per core
NPAIRS = NTILES // 2    # 4 supertiles of [128, 2, 512]
NJC = NV // P           # 4 codebook chunks of 128
KAUG = D + 3            # x(64) + 1 + xsq_hi + xsq_lo

_CACHE = {}


def _make_nc():
    return bacc.Bacc(
        "TRN2",
        target_bir_lowering=False,
        debug=False,
        enable_asserts=False,
        num_devices=NCORES,
    )


def _declare_io(nc):
    f32 = mybir.dt.float32
    f32r = mybir.dt.float32r
    return {
        "xaugT": nc.dram_tensor("xaugT", [KAUG, BPC], f32r, kind="ExternalInput").ap(),
        "paugT": nc.dram_tensor("paugT", [KAUG, NV], f32r, kind="ExternalInput").ap(),
        "dist": nc.dram_tensor("dist", [BPC, NV], f32, kind="ExternalOutput").ap(),
        "colfin": nc.dram_tensor("colfin", [P, NJC], f32, kind="ExternalOutput").ap(),
        "rowmin": nc.dram_tensor("rowmin", [P, NTILES], f32, kind="ExternalOutput").ap(),
    }


def _emit_body(nc, io, singles, dists, psums, apsums, xh_sb, paugT_sb):
    """One full pass over the core's 1024 x 512 distance block.

    The transposed-layout (col-min) rounds are interleaved with the main
    rounds so VectorE - the bottleneck engine - is never starved waiting
    on a serial tail of matmuls."""
    f32 = mybir.dt.float32
    mn = mybir.AluOpType.min
    rowmin_sb = singles.tile([P, NTILES], f32)
    colfin_sb = singles.tile([P, NJC], f32)

    for g in range(2):
        # transposed round FIRST: jc = 2g, 2g+1 in one 4-bank PSUM quad
        # -> ONE col-min reduce per two codebook chunks.  Emitting it
        # before the main rounds lets VectorE start ~2us earlier in the
        # cold single-shot execution (the col-min reduce depends only on
        # matmuls, not on the activation-table load + sqrt chain).
        apsum = apsums.tile([P, 2, 2, NV], f32)   # [128 j, jc-pair, batch-half, 512]
        for h in range(2):       # h-major: the first two matmuls need only
            for a in range(2):   # the x half-0 load (earlier cold start)
                jc = 2 * g + a
                nc.tensor.matmul(
                    apsum[:, a, h, :],
                    paugT_sb[:, jc * P:(jc + 1) * P],
                    xh_sb[h][:],
                    start=True,
                    stop=True,
                )
        nc.vector.tensor_reduce(
            colfin_sb[:, 2 * g:2 * g + 2],
            apsum[:],
            axis=mybir.AxisListType.XY,
            op=mn,
        )

        # main rounds: two D2[i, j] pairs -> sqrt into one [128, 4, 512]
        # quad, so the row-min is ONE DVE reduce per 4 tiles (amortizes
        # the ~150-cycle per-op DVE overhead)
        dist_quad = dists.tile([P, 4, NV], f32)
        for q2 in range(2):
            psum_pair = psums.tile([P, 2, NV], f32)
            for h in range(2):
                t = 4 * g + 2 * q2 + h
                nc.tensor.matmul(
                    psum_pair[:, h, :],
                    xh_sb[t // 4][:, (t % 4) * P:(t % 4 + 1) * P],
                    paugT_sb[:],
                    start=True,
                    stop=True,
                )
            nc.scalar.sqrt(dist_quad[:, 2 * q2:2 * q2 + 2, :], psum_pair[:])
        nc.sync.dma_start(
            out=io["dist"][4 * g * P:(4 * g + 4) * P, :].rearrange(
                "(b p) j -> p b j", b=4),
            in_=dist_quad[:],
        )
        # r2 partials: min over the 512 codebook entries per row, on the
        # sqrt'd SBUF quad (min commutes with sqrt)
        nc.vector.tensor_reduce(
            rowmin_sb[:, 4 * g:4 * g + 4],
            dist_quad[:],
            axis=mybir.AxisListType.X,
            op=mn,
        )

    nc.scalar.dma_start(out=io["colfin"][:], in_=colfin_sb[:])
    nc.sync.dma_start(out=io["rowmin"][:], in_=rowmin_sb[:])


def _build_program(outer_loop=None, inner_unroll=1):
    """outer_loop=None -> single-pass production program.
    outer_loop=K -> For_i hardware loop with inner_unroll python-unrolled
    passes per iteration (timing amplification)."""
    f32r = mybir.dt.float32r
    nc = _make_nc()
    io = _declare_io(nc)

    with tile.TileContext(nc) as tc:
        with (
            tc.tile_pool(name="consts", bufs=1) as consts,
            tc.tile_pool(name="singles", bufs=2) as singles,
            tc.tile_pool(name="dists", bufs=2) as dists,
            tc.tile_pool(name="psums", bufs=2, space="PSUM") as psums,
            tc.tile_pool(name="apsums", bufs=1, space="PSUM") as apsums,
        ):
            # x halves as separate tiles: the first matmuls start as soon
            # as half 0 lands, and the transposed rounds address halves
            xh0_sb = consts.tile([KAUG, BPC // 2], f32r)
            xh1_sb = consts.tile([KAUG, BPC // 2], f32r)
            xh_sb = [xh0_sb, xh1_sb]
            paugT_sb = consts.tile([KAUG, NV], f32r)
            # two HWDGE queues (SP + Activation): parallelize input loads
            # (xh0 alone on sync; paugT + xh1 share the activation queue)
            nc.scalar.dma_start(out=paugT_sb[:], in_=io["paugT"][:])
            nc.sync.dma_start(out=xh_sb[0][:], in_=io["xaugT"][:, :BPC // 2])
            nc.scalar.dma_start(out=xh_sb[1][:], in_=io["xaugT"][:, BPC // 2:])

            if outer_loop is None:
                _emit_body(nc, io, singles, dists, psums, apsums,
                           xh_sb, paugT_sb)
            else:
                with tc.For_i(0, outer_loop, 1):
                    for _ in range(inner_unroll):
                        _emit_body(nc, io, singles, dists, psums, apsums,
                                   xh_sb, paugT_sb)

    nc.compile()
    return nc


def _get_program():
    if "nc" not in _CACHE:
        _CACHE["nc"] = _build_program()
    return _CACHE["nc"]


def _prep_inputs(x, p):
    x = np.ascontiguousarray(np.asarray(x, dtype=np.float32))
    p = np.ascontiguousarray(np.asarray(p, dtype=np.float32))
    xsq = np.einsum("id,id->i", x.astype(np.float64), x.astype(np.float64))
    xsq = xsq.astype(np.float32)
    psq = np.einsum("jd,jd->j", p.astype(np.float64), p.astype(np.float64))
    xsq_hi = xsq.astype(np.dtype("bfloat16") if hasattr(np, "bfloat16")
                        else np.float32)
    # bf16 rounding via ml_dtypes if available, else manual truncation
    try:
        import ml_dtypes
        xsq_hi = xsq.astype(ml_dtypes.bfloat16).astype(np.float32)
    except ImportError:
        xsq_hi = (xsq.view(np.uint32) & np.uint32(0xFFFF0000)).view(np.float32)
    xsq_lo = xsq - xsq_hi
    xaugT = np.empty((KAUG, BATCH), np.float32)
    xaugT[:D] = x.T
    xaugT[D] = 1.0
    xaugT[D + 1] = xsq_hi
    xaugT[D + 2] = xsq_lo
    paugT = np.empty((KAUG, NV), np.float32)
    paugT[:D] = -2.0 * p.T
    paugT[D] = psq.astype(np.float32)
    paugT[D + 1] = 1.0
    paugT[D + 2] = 1.0
    in_maps = []
    for c in range(NCORES):
        in_maps.append({
            "xaugT": np.ascontiguousarray(xaugT[:, c * BPC:(c + 1) * BPC]),
            "paugT": paugT,
        })
    return in_maps


def _run(x, p, trace=False, nc=None, **kwargs):
    if nc is None:
        nc = _get_program()
    in_maps = _prep_inputs(x, p)
    return run_bass_kernel_spmd(
        nc, in_maps, core_ids=list(range(NCORES)), trace=trace, **kwargs
    )


def _assemble(results):
    dist_full = np.empty((BATCH, NV), np.float32)
    colfins = np.empty((NCORES, P, NJC), np.float32)
    rowmins = np.empty((NCORES, P, NTILES), np.float32)
    for c in range(NCORES):
        r = results[c]
        dist_full[c * BPC:(c + 1) * BPC] = r["dist"]
        colfins[c] = r["colfin"]
        rowmins[c] = r["rowmin"]
    # cross-core all-reduce-min over batch (D2 domain), then codebook mean
    r1 = np.float32(np.mean(np.sqrt(colfins.min(axis=0).astype(np.float64))))
    # per-sample min is already complete locally (dist domain); batch mean
    r2 = np.float32(np.mean(rowmins.astype(np.float64)))
    return dist_full, r1, r2


def kernel(x, trainable_p):
    try:
        res = _run(x, trainable_p)
    except Exception:
        # transient device wedge (e.g. NRT_EXEC_UNIT_UNRECOVERABLE):
        # rebuild the program and retry once on a fresh session
        _CACHE.clear()
        res = _run(x, trainable_p)
    return _assemble(res.results)


# revision 33
# speedup vs baseline: 1.2653x; 1.2653x over previous
"""Trainium2 Bass kernel for nn_EuclideanEmbedding (vq_codebook).

reference:
    distances = cdist(x, p)                      # (8192, 512)
    r1 = mean_j min_i distances[i, j]            # scalar
    r2 = mean_i min_j distances[i, j]            # scalar

Strategy (8 NeuronCores):
  - Shard x along batch: 1024 rows/core; replicate the (512, 64) codebook.
  - Per core, squared distances via float32r matmuls (1 row/cycle on the
    PE vs 4 for plain fp32), one per 128-row tile, K = 67:
      D2 = [x, 1, hi, lo] @ [-2 p, ||p||^2, 1, 1]^T
    where hi = bf16(||x||^2), lo = ||x||^2 - hi.  f32r rounds operands to
    ~11 mantissa bits; splitting the large ||x||^2 term into a bf16 part
    (exactly f32r-representable) plus a small residual keeps the D2
    error at the x.p noise floor instead of ||x||^2 * 2^-12.
  - Tiles are processed in pairs ([128, 2, 512] PSUM supertiles);
    ScalarE takes sqrt (PSUM->SBUF, one op per pair); DMA streams the
    distance shard out (contiguous 256KB blocks).
  - VectorE r2 partials: per-row min over the codebook on PSUM D2.
  - VectorE r1 partials: a SECOND set of matmuls emits the transposed
    layout D2T[j, batch] (the tensor engine has spare capacity), so the
    min-over-batch is also a free-dim reduce - no elementwise min chain,
    no on-chip transposes. Only a [128, 4] vector leaves each core.
  - Host combines the per-core partial minima (the cross-device min /
    mean all-reduce of the sharding hint) while unsharding.
"""

import numpy as np

import concourse.bacc as bacc
import concourse.tile as tile
from concourse import mybir
from concourse.bass_utils import run_bass_kernel_spmd

BATCH = 8192
NV = 512          # codebook vectors
D = 64            # latent dim
NCORES = 8
BPC = BATCH // NCORES   # 1024 batch rows per core
P = 128                 # partitions
NTILES = BPC // P       # 8 tiles of 128 batch rows per core
NPAIRS = NTILES // 2    # 4 supertiles of [128, 2, 512]
NJC = NV // P           # 4 codebook chunks of 128
KAUG = D + 3            # x(64) + 1 + xsq_hi + xsq_lo

_CACHE = {}


def _make_nc():
    return bacc.Bacc(
        "TRN2",
        target_bir_lowering=False,
        debug=False,
        enable_asserts=False,
        num_devices=NCORES,
    )


def _declare_io(nc):
    f32 = mybir.dt.float32
    f32r = mybir.dt.float32r
    return {
        "xaugT": nc.dram_tensor("xaugT", [KAUG, BPC], f32r, kind="ExternalInput").ap(),
        "paugT": nc.dram_tensor("paugT", [KAUG, NV], f32r, kind="ExternalInput").ap(),
        "dist": nc.dram_tensor("dist", [BPC, NV], f32, kind="ExternalOutput").ap(),
        "colfin": nc.dram_tensor("colfin", [P, NJC], f32, kind="ExternalOutput").ap(),
        "rowmin": nc.dram_tensor("rowmin", [P, NTILES], f32, kind="ExternalOutput").ap(),
    }


def _emit_body(nc, io, singles, dists, psums, apsums, apsums2, scr, xh_sb, paugT_sb):
    """One full pass over the core's 1024 x 512 distance block.

    The transposed-layout (col-min) rounds are interleaved with the main
    rounds so VectorE - the bottleneck engine - is never starved waiting
    on a serial tail of matmuls."""
    f32 = mybir.dt.float32
    mn = mybir.AluOpType.min
    rowmin_sb = singles.tile([P, NTILES], f32)
    colfin_sb = singles.tile([P, NJC], f32)

    for g in range(2):
        # transposed round FIRST: jc = 2g, 2g+1 in one 4-bank PSUM quad
        # -> ONE col-min reduce per two codebook chunks.  Emitting it
        # before the main rounds lets VectorE start ~2us earlier in the
        # cold single-shot execution (the col-min reduce depends only on
        # matmuls, not on the activation-table load + sqrt chain).
        apsum = apsums.tile([P, 2, 2, NV], f32)   # [128 j, jc-pair, batch-half, 512]
        for h in range(2):       # h-major: the first two matmuls need only
            for a in range(2):   # the x half-0 load (earlier cold start)
                jc = 2 * g + a
                nc.tensor.matmul(
                    apsum[:, a, h, :],
                    paugT_sb[:, jc * P:(jc + 1) * P],
                    xh_sb[h][:],
                    start=True,
                    stop=True,
                )
        nc.vector.tensor_reduce(
            colfin_sb[:, 2 * g:2 * g + 2],
            apsum[:],
            axis=mybir.AxisListType.XY,
            op=mn,
        )

        # main rounds: two D2[i, j] pairs -> sqrt into one [128, 4, 512]
        # quad, so the row-min is ONE DVE reduce per 4 tiles (amortizes
        # the ~150-cycle per-op DVE overhead)
        dist_quad = dists.tile([P, 4, NV], f32)
        for q2 in range(2):
            psum_pair = psums.tile([P, 2, NV], f32)
            for h in range(2):
                t = 4 * g + 2 * q2 + h
                nc.tensor.matmul(
                    psum_pair[:, h, :],
                    xh_sb[t // 4][:, (t % 4) * P:(t % 4 + 1) * P],
                    paugT_sb[:],
                    start=True,
                    stop=True,
                )
            nc.scalar.sqrt(dist_quad[:, 2 * q2:2 * q2 + 2, :], psum_pair[:])
        nc.sync.dma_start(
            out=io["dist"][4 * g * P:(4 * g + 4) * P, :].rearrange(
                "(b p) j -> p b j", b=4),
            in_=dist_quad[:],
        )
        # r2 partials: min over the 512 codebook entries per row, on the
        # sqrt'd SBUF quad (min commutes with sqrt)
        nc.vector.tensor_reduce(
            rowmin_sb[:, 4 * g:4 * g + 4],
            dist_quad[:],
            axis=mybir.AxisListType.X,
            op=mn,
        )

    nc.scalar.dma_start(out=io["colfin"][:], in_=colfin_sb[:])
    nc.sync.dma_start(out=io["rowmin"][:], in_=rowmin_sb[:])


def _build_program(outer_loop=None, inner_unroll=1):
    """outer_loop=None -> single-pass production program.
    outer_loop=K -> For_i hardware loop with inner_unroll python-unrolled
    passes per iteration (timing amplification)."""
    f32r = mybir.dt.float32r
    nc = _make_nc()
    io = _declare_io(nc)

    with tile.TileContext(nc) as tc:
        with (
            tc.tile_pool(name="consts", bufs=1) as consts,
            tc.tile_pool(name="singles", bufs=2) as singles,
            tc.tile_pool(name="dists", bufs=2) as dists,
            tc.tile_pool(name="psums", bufs=2, space="PSUM") as psums,
            tc.tile_pool(name="apsums", bufs=1, space="PSUM") as apsums,
            tc.tile_pool(name="apsums2", bufs=1, space="PSUM") as apsums2,
            tc.tile_pool(name="scr", bufs=2) as scr,
        ):
            # x halves as separate tiles: the first matmuls start as soon
            # as half 0 lands, and the transposed rounds address halves
            xh0_sb = consts.tile([KAUG, BPC // 2], f32r)
            xh1_sb = consts.tile([KAUG, BPC // 2], f32r)
            xh_sb = [xh0_sb, xh1_sb]
            paugT_sb = consts.tile([KAUG, NV], f32r)
            # two HWDGE queues (SP + Activation): parallelize input loads
            # (xh0 alone on sync; paugT + xh1 share the activation queue)
            nc.scalar.dma_start(out=paugT_sb[:], in_=io["paugT"][:])
            nc.sync.dma_start(out=xh_sb[0][:], in_=io["xaugT"][:, :BPC // 2])
            nc.scalar.dma_start(out=xh_sb[1][:], in_=io["xaugT"][:, BPC // 2:])

            if outer_loop is None:
                _emit_body(nc, io, singles, dists, psums, apsums, apsums2, scr,
                           xh_sb, paugT_sb)
            else:
                with tc.For_i(0, outer_loop, 1):
                    for _ in range(inner_unroll):
                        _emit_body(nc, io, singles, dists, psums, apsums,
                                   apsums2, scr, xh_sb, paugT_sb)

    nc.compile()
    return nc


def _get_program():
    if "nc" not in _CACHE:
        _CACHE["nc"] = _build_program()
    return _CACHE["nc"]


def _prep_inputs(x, p):
    x = np.ascontiguousarray(np.asarray(x, dtype=np.float32))
    p = np.ascontiguousarray(np.asarray(p, dtype=np.float32))
    xsq = np.einsum("id,id->i", x.astype(np.float64), x.astype(np.float64))
    xsq = xsq.astype(np.float32)
    psq = np.einsum("jd,jd->j", p.astype(np.float64), p.astype(np.float64))
    xsq_hi = xsq.astype(np.dtype("bfloat16") if hasattr(np, "bfloat16")
                        else np.float32)
    # bf16 rounding via ml_dtypes if available, else manual truncation
    try:
        import ml_dtypes
        xsq_hi = xsq.astype(ml_dtypes.bfloat16).astype(np.float32)
    except ImportError:
        xsq_hi = (xsq.view(np.uint32) & np.uint32(0xFFFF0000)).view(np.float32)
    xsq_lo = xsq - xsq_hi
    xaugT = np.empty((KAUG, BATCH), np.float32)
    xaugT[:D] = x.T
    xaugT[D] = 1.0
    xaugT[D + 1] = xsq_hi
    xaugT[D + 2] = xsq_lo
    paugT = np.empty((KAUG, NV), np.float32)
    paugT[:D] = -2.0 * p.T
    paugT[D] = psq.astype(np.float32)
    paugT[D + 1] = 1.0
    paugT[D + 2] = 1.0
    in_maps = []
    for c in range(NCORES):
        in_maps.append({
            "xaugT": np.ascontiguousarray(xaugT[:, c * BPC:(c + 1) * BPC]),
            "paugT": paugT,
        })
    return in_maps


def _run(x, p, trace=False, nc=None, **kwargs):
    if nc is None:
        nc = _get_program()
    in_maps = _prep_inputs(x, p)
    return run_bass_kernel_spmd(
        nc, in_maps, core_ids=list(range(NCORES)), trace=trace, **kwargs
    )


def _assemble(results):
    dist_full = np.empty((BATCH, NV), np.float32)
    colfins = np.empty((NCORES, P, NJC), np.float32)
    rowmins = np.empty((NCORES, P, NTILES), np.float32)
    for c in range(NCORES):
        r = results[c]
        dist_full[c * BPC:(c + 1) * BPC] = r["dist"]
        colfins[c] = r["colfin"]
        rowmins[c] = r["rowmin"]
    # cross-core all-reduce-min over batch (D2 domain), then codebook mean
    r1 = np.float32(np.mean(np.sqrt(colfins.min(axis=0).astype(np.float64))))
    # per-sample min is already complete locally (dist domain); batch mean
    r2 = np.float32(np.mean(rowmins.astype(np.float64)))
    return dist_full, r1, r2


def kernel(x, trainable_p):
    try:
        res = _run(x, trainable_p)
    except Exception:
        # transient device wedge (e.g. NRT_EXEC_UNIT_UNRECOVERABLE):
        # rebuild the program and retry once on a fresh session
        _CACHE.clear()
        res = _run(x, trainable_p)
    return _assemble(res.results)
